# revision 1
# baseline (speedup 1.0000x reference)
"""Chain-CRF negative log-likelihood on 8 Trainium2 NeuronCores (Bass/Tile).

Strategy (pure data parallelism, batch 512 -> 64 per core):
  The CRF forward algorithm is run in scaled exp-space ("scaled forward
  algorithm"): p_t = (Texp^T p_{t-1}) o exp(feats_t), where Texp = exp(log_transitions).
  Each step is one small PE matmul (stationary [48,65] transition matrix
  augmented with a Tend column at position 64 — engine APs must start at
  mod-32 partitions — whose output row doubles as the per-step partition
  snapshot w_t AND the periodic renormalization divisor) followed by one DVE
  elementwise multiply.

  Variable sequence lengths are handled without any per-step masking:
    - forward half (t = 0..127): snapshots w_t = sum_i p_t[i]*Tend[i] land for
      free in the matmul output (row 48); sequences with L <= 127 read their
      answer from w_{L-1}.
    - backward half (t = 255..127) runs CONCURRENTLY (second dependency chain
      keeps both engines busy): beta recursion with a rank-1 "injection"
      matmul (PSUM accumulate) that starts sequence b's backward pass exactly
      at t = L_b - 1. Sequences with L >= 128 combine at the midpoint:
      partition = log sum_i alphahat_127[i] * betahat_127[i] + scales.
  Renormalization every 8 steps divides the state by (approximately) its
  colsum; the actually-applied factors r are logged at the very end (one Ln +
  one prefix-scan) so no transcendental sits on the critical path.

  Gold path score: transition terms via a host-built integer count matrix
  (device matmul against exp->log transition values read from the lt input);
  emission terms are selected host-side (HW indirect DMA cannot do
  per-element gathers) and summed on device. All floating-point arithmetic
  happens on device; the host does layout transforms, integer index/mask/
  count preprocessing, and the emission value selection.
"""

import os

os.environ.setdefault("NEURON_CC_FLAGS", "")

import numpy as np
from contextlib import ExitStack

import concourse.bass as bass
import concourse.tile as tile
from concourse import bacc, mybir
from concourse.bass_utils import run_bass_kernel_spmd

# ---- problem constants (hardcoded per contract) ----
B = 512
M = 256
T = 48          # n_tags
ROOT = 46
END = 47
NC = 8
BL = B // NC    # 64 sequences per core
HALF = M // 2   # 128
FTG_N = T * M * BL + BL   # transposed feats + zero pad tail

F32 = mybir.dt.float32
AF = mybir.ActivationFunctionType
ALU = mybir.AluOpType

_PROGRAM = None


def _build_program():
    nc = bacc.Bacc(
        "TRN2",
        target_bir_lowering=False,
        debug=False,
        enable_asserts=False,
        num_devices=NC,
    )

    ftg = nc.dram_tensor("ftg", [FTG_N], F32, kind="ExternalInput").ap()
    lt = nc.dram_tensor("lt", [T, T], F32, kind="ExternalInput").ap()
    emitv = nc.dram_tensor("emitv", [BL, M], F32, kind="ExternalInput").ap()
    inj = nc.dram_tensor("inj", [1, 129 * BL], F32, kind="ExternalInput").ap()
    selw = nc.dram_tensor("selw", [BL, HALF], F32, kind="ExternalInput").ap()
    selk = nc.dram_tensor("selk", [BL, 16], F32, kind="ExternalInput").ap()
    sbm = nc.dram_tensor("sbm", [BL, 16], F32, kind="ExternalInput").ap()
    selfb = nc.dram_tensor("selfb", [BL, 2], F32, kind="ExternalInput").ap()
    cmat = nc.dram_tensor("cmat", [128, 18 * BL], F32, kind="ExternalInput").ap()
    ident = nc.dram_tensor("ident", [T, T], F32, kind="ExternalInput").ap()
    out = nc.dram_tensor("out", [BL, 1], F32, kind="ExternalOutput").ap()
    dbg = None
    if os.environ.get("K_DEBUG_OUT"):
        dbg = nc.dram_tensor("dbg", [BL, 8], F32, kind="ExternalOutput").ap()
        dbg2 = nc.dram_tensor("dbg2", [BL, 48], F32, kind="ExternalOutput").ap()

    with tile.TileContext(nc) as tc, ExitStack() as ctx:
        _emit_body(ctx, tc, ftg, lt, emitv, inj, selw, selk, sbm, selfb, cmat,
                   ident, out, dbg, dbg2 if dbg is not None else None)
    nc.finalize()  # runs Bacc.compile(): register allocation etc.
    return nc


def _emit_body(ctx, tc, ftg, lt, emitv, inj, selw, selk, sbm, selfb, cmat,
               ident, out, dbg=None, dbg2=None):
    nc = tc.nc

    const = ctx.enter_context(tc.tile_pool(name="const", bufs=1))
    raws = ctx.enter_context(tc.tile_pool(name="raws", bufs=6))
    slab = ctx.enter_context(tc.tile_pool(name="slab", bufs=1))
    pstate = ctx.enter_context(tc.tile_pool(name="pstate", bufs=6))
    ustate = ctx.enter_context(tc.tile_pool(name="ustate", bufs=6))
    smalls = ctx.enter_context(tc.tile_pool(name="smalls", bufs=8))
    endp = ctx.enter_context(tc.tile_pool(name="endp", bufs=2))
    fps_pool = ctx.enter_context(tc.tile_pool(name="fps", bufs=2, space="PSUM"))
    bps_pool = ctx.enter_context(tc.tile_pool(name="bps", bufs=2, space="PSUM"))
    bc_pool = ctx.enter_context(tc.tile_pool(name="bc", bufs=1, space="PSUM"))
    tmp_ps = ctx.enter_context(tc.tile_pool(name="tmpps", bufs=1, space="PSUM"))
    pair_pool = ctx.enter_context(tc.tile_pool(name="pairps", bufs=1, space="PSUM"))
    dram = ctx.enter_context(tc.tile_pool(name="dram", bufs=1, space="DRAM"))

    # ---------------- constants ----------------
    lt_sb = const.tile([T, T], F32)
    nc.sync.dma_start(lt_sb[:], lt[:])
    ident_sb = const.tile([T, T], F32)
    nc.sync.dma_start(ident_sb[:], ident[:])

    # TA_f [48, 65]: cols 0..47 = exp(lt); col 64 = Tend = exp(lt[:, END]).
    # The aux row must land at a mod-32 psum partition (engine APs may only
    # start at partitions 0/32/64/96), hence col 64. The w row doubles as the
    # forward renorm divisor.
    ta_f = const.tile([T, 65], F32)
    nc.gpsimd.memset(ta_f[:], 0.0)
    nc.scalar.activation(ta_f[:, 0:T], lt_sb[:], AF.Exp)
    nc.scalar.copy(ta_f[:, 64:65], ta_f[:, END:END + 1])

    # TA_b [48, 65]: cols 0..47 = exp(lt)^T; col 64 = 1 (colsum row)
    tr_ps = tmp_ps.tile([T, T], F32, space="PSUM", tag="tmp")
    nc.tensor.transpose(out=tr_ps[:], in_=ta_f[:, 0:T], identity=ident_sb[:])
    ta_b = const.tile([T, 65], F32)
    nc.gpsimd.memset(ta_b[:], 0.0)
    nc.vector.tensor_copy(ta_b[:, 0:T], tr_ps[:])
    nc.gpsimd.memset(ta_b[:, 64:65], 1.0)

    # Tend as a row [1, 48] (injection lhsT) and RootExp [48, 1]
    tendlog = const.tile([1, T], F32)
    nc.sync.dma_start(tendlog[:], lt[:, END:END + 1].rearrange("a b -> b a"))
    tendrow = const.tile([1, 65], F32)
    nc.gpsimd.memset(tendrow[:], 0.0)
    nc.gpsimd.memset(tendrow[:, 64:65], 1.0)
    nc.scalar.activation(tendrow[:, 0:T], tendlog[:], AF.Exp)
    rootlog = const.tile([T, 1], F32)
    nc.sync.dma_start(rootlog[:], lt[ROOT:ROOT + 1, :].rearrange("a b -> b a"))
    rootexp = const.tile([T, 1], F32)
    nc.scalar.activation(rootexp[:], rootlog[:], AF.Exp)

    onesrow = const.tile([1, T], F32)
    nc.gpsimd.memset(onesrow[:], 1.0)
    onescol = const.tile([T, 1], F32)
    nc.gpsimd.memset(onescol[:], 1.0)

    # lt as [128, 18] column chunks (for the gold transition matmuls)
    ltsb = const.tile([128, 18], F32)
    nc.sync.dma_start(ltsb[:], lt.rearrange("a b -> (a b)").rearrange("(k p) -> p k", p=128))

    cmat_sb = const.tile([128, 18 * BL], F32)
    nc.sync.dma_start(cmat_sb[:], cmat[:])
    emitv_sb = const.tile([BL, M], F32)
    nc.sync.dma_start(emitv_sb[:], emitv[:])
    inj_sb = const.tile([1, 129 * BL], F32)
    nc.sync.dma_start(inj_sb[:], inj[:])
    selw_sb = const.tile([BL, HALF], F32)
    nc.sync.dma_start(selw_sb[:], selw[:])
    selk_sb = const.tile([BL, 16], F32)
    nc.sync.dma_start(selk_sb[:], selk[:])
    sbm_sb = const.tile([BL, 16], F32)
    nc.sync.dma_start(sbm_sb[:], sbm[:])
    selfb_sb = const.tile([BL, 2], F32)
    nc.sync.dma_start(selfb_sb[:], selfb[:])

    # histories: staged to DRAM during the scan (t-major), loaded back
    # transposed at the end via DRAM-side strided APs
    wdram = dram.tile([1, HALF * BL], F32, tag="wdram")
    rfdram = dram.tile([1, 16 * BL], F32, tag="rfdram")
    rbdram = dram.tile([1, 16 * BL], F32, tag="rbdram")
    zdram = dram.tile([1, BL], F32, tag="zdram")
    zeros16 = const.tile([BL, 16], F32)
    nc.gpsimd.memset(zeros16[:], 0.0)
    zrow64 = const.tile([1, BL], F32)
    nc.gpsimd.memset(zrow64[:], 0.0)
    # wdram col 127 is never produced by the scan; keep it defined
    nc.sync.dma_start(wdram[:, 127 * BL:128 * BL], zrow64[:])

    _trunc = int(os.environ.get("K_TRUNC", "99"))
    if _trunc <= 1:
        dummy = const.tile([BL, 1], F32)
        nc.gpsimd.memset(dummy[:], 0.0)
        nc.sync.dma_start(out[:], dummy[:])
        return

    # ---------------- gold score (independent of the scan) ----------------
    emitsum = const.tile([BL, 1], F32)
    nc.vector.tensor_reduce(emitsum[:], emitv_sb[:], axis=mybir.AxisListType.X,
                            op=ALU.add)

    pair_ps = pair_pool.tile([BL, 1], F32, space="PSUM")
    for k in range(18):
        nc.tensor.matmul(
            out=pair_ps[:],
            lhsT=cmat_sb[:, k * BL:(k + 1) * BL],
            rhs=ltsb[:, k:k + 1],
            start=(k == 0),
            stop=(k == 17),
        )

    if _trunc <= 2:
        dummy = const.tile([BL, 1], F32)
        nc.vector.tensor_copy(dummy[:], pair_ps[:])
        nc.sync.dma_start(out[:], dummy[:])
        return

    # ---------------- exp(feats) slabs ----------------
    fT = ftg[0:T * M * BL].rearrange("(j tb) -> j tb", j=T)   # [48, 16384]
    ef = slab.tile([T, HALF * BL], F32)   # t in [0, 128)
    eb = slab.tile([T, HALF * BL], F32)   # t in [128, 256)
    # interleave chunk order from both ends: fwd consumes Ef from t=0 up,
    # bwd consumes Eb from t=255 down — both chains get their first tiles
    # early; the very first/last pieces are split small so step 1 of each
    # chain starts as soon as possible
    pieces = [(0, 128), (16384 - 128, 128), (128, 384), (16384 - 512, 384)]
    for k in range(1, 16):
        pieces += [(k * 512, 512), (16384 - (k + 1) * 512, 512)]
    for off, ln in pieces:
        raw = raws.tile([T, 512], F32, tag="raw")
        nc.sync.dma_start(raw[:, 0:ln], fT[:, off:off + ln])
        dst, o = (ef, off) if off < 8192 else (eb, off - 8192)
        nc.scalar.activation(dst[:, o:o + ln], raw[:, 0:ln], AF.Exp)

    def e_f(t):
        return ef[:, t * BL:(t + 1) * BL]

    def e_b(t):
        return eb[:, (t - HALF) * BL:(t - HALF + 1) * BL]

    FWD_RENORMS = [8 * k for k in range(1, 16)] + [127]      # after-mult steps t
    BWD_RENORMS = [4 + 8 * k for k in range(16)]             # bwd step index s

    def renorm_prep(d_row, r_hist, rk, tagname):
        """Compute broadcast reciprocal of the psum divisor row; record the
        applied factor into r_hist dram col rk. Emitted BEFORE the state
        multiply so dg/recip (DVE) and the broadcast (PE) overlap it."""
        dg = smalls.tile([1, BL], F32, tag=f"dg{tagname}")
        # 1e-6 floor: bounds the amplification when a backward injection lands
        # exactly on a renorm step (pre-injection colsum is 0 there)
        nc.vector.tensor_scalar_max(dg[:], d_row, 1e-6)
        r = smalls.tile([1, BL], F32, tag=f"r{tagname}")
        nc.vector.reciprocal_approx_fast(out=r[:], in_=dg[:])
        nc.sync.dma_start(r_hist[:, rk * BL:(rk + 1) * BL], r[:])
        bcp = bc_pool.tile([T, BL], F32, space="PSUM", tag=f"bc{tagname}")
        nc.tensor.matmul(out=bcp[:], lhsT=onesrow[:], rhs=r[:], start=True, stop=True)
        return bcp

    def renorm_apply(state, bcp, tagname):
        scaled = (pstate if tagname == "f" else ustate).tile(
            [T, BL], F32, tag=f"sc{tagname}")
        nc.vector.tensor_tensor(out=scaled[:], in0=state[:], in1=bcp[:], op=ALU.mult)
        return scaled

    # ---------------- the two scan chains, interleaved ----------------
    # fwd state
    p_prev = pstate.tile([T, BL], F32, tag="p")
    nc.vector.tensor_scalar_mul(p_prev[:], e_f(0), rootexp[:, 0:1])
    # bwd state: u_255 = E_255 o (inj_255 x Tend)
    bps = bps_pool.tile([65, 512], F32, space="PSUM")
    nc.tensor.matmul(out=bps[0:65, 0:BL], lhsT=tendrow[:],
                     rhs=inj_sb[:, 128 * BL:129 * BL], start=True, stop=True)
    u_prev = ustate.tile([T, BL], F32, tag="u")
    nc.vector.tensor_tensor(out=u_prev[:], in0=bps[0:T, 0:BL], in1=e_b(255), op=ALU.mult)

    fps = None
    pf127 = None
    frk = 0
    brk = 0
    beta127 = None

    for i in range(1, 129):
        # ---- forward step t = i (runs t = 1..127) ----
        t = i
        if t <= 127:
            blk, col = (t - 1) // 8, (t - 1) % 8
            if col == 0:
                fps = fps_pool.tile([65, 512], F32, space="PSUM")
            nc.tensor.matmul(out=fps[:, col * BL:(col + 1) * BL], lhsT=ta_f[:],
                             rhs=p_prev[:], start=True, stop=True)
            bcp = None
            if t in FWD_RENORMS:
                bcp = renorm_prep(fps[64:65, col * BL:(col + 1) * BL],
                                  rfdram, frk, "f")
                frk += 1
            p_cur = pstate.tile([T, BL], F32, tag="p")
            nc.vector.tensor_tensor(out=p_cur[:], in0=fps[0:T, col * BL:(col + 1) * BL],
                                    in1=e_f(t), op=ALU.mult)
            if bcp is not None:
                p_cur = renorm_apply(p_cur, bcp, "f")
            if col == 7 or t == 127:
                ncols = col + 1
                wst = smalls.tile([1, 512], F32, tag="wst")
                # scale by 2^-48 so ln(w) stays inside ScalarE Ln's |x|<=2^64
                # domain; compensated after the Ln
                nc.scalar.activation(wst[:, 0:ncols * BL], fps[64:65, 0:ncols * BL],
                                     AF.Copy, scale=2.0 ** -29)
                nc.sync.dma_start(wdram[:, blk * 8 * BL:(blk * 8 + ncols) * BL],
                                  wst[:, 0:ncols * BL])
            if t == 127:
                pf127 = p_cur
            p_prev = p_cur

        # ---- backward step s = i (runs s = 1..128), tau = 255 - s ----
        s = i
        tau = 255 - s
        blk, col = s // 8, s % 8
        if col == 0:
            bps = bps_pool.tile([65, 512], F32, space="PSUM")
        # inject FIRST (start=True): it depends only on constants, so it can
        # fire long before u_prev is ready — keeps the K=1 matmul off the
        # backward chain's critical path (addition into PSUM commutes)
        nc.tensor.matmul(out=bps[:, col * BL:(col + 1) * BL], lhsT=tendrow[:],
                         rhs=inj_sb[:, (tau - 127) * BL:(tau - 126) * BL],
                         start=True, stop=False)
        nc.tensor.matmul(out=bps[:, col * BL:(col + 1) * BL], lhsT=ta_b[:],
                         rhs=u_prev[:], start=False, stop=True)
        if s <= 127:
            bcpb = None
            if s in BWD_RENORMS:
                bcpb = renorm_prep(bps[64:65, col * BL:(col + 1) * BL],
                                   rbdram, brk, "b")
                brk += 1
            u_cur = ustate.tile([T, BL], F32, tag="u")
            nc.vector.tensor_tensor(out=u_cur[:], in0=bps[0:T, col * BL:(col + 1) * BL],
                                    in1=e_b(tau), op=ALU.mult)
            if bcpb is not None:
                u_cur = renorm_apply(u_cur, bcpb, "b")
            u_prev = u_cur
        else:
            beta127 = bps[0:T, col * BL:(col + 1) * BL]

    if _trunc <= 3:
        dummy = const.tile([BL, 1], F32)
        nc.vector.tensor_copy(dummy[:], pf127[:, 0:1])
        nc.sync.dma_start(out[:], dummy[:])
        return

    # ---------------- combine & finalize ----------------
    zz = endp.tile([T, BL], F32)
    nc.vector.tensor_tensor(out=zz[:], in0=pf127[:], in1=beta127, op=ALU.mult)
    z_ps = tmp_ps.tile([1, BL], F32, space="PSUM", tag="tmp")
    nc.tensor.matmul(out=z_ps[:], lhsT=onescol[:], rhs=zz[:], start=True, stop=True)
    zrow = endp.tile([1, BL], F32)
    nc.vector.tensor_scalar_max(zrow[:], z_ps[:], 1e-37)
    nc.sync.dma_start(zdram[:], zrow[:])
    zT = endp.tile([BL, 1], F32)
    nc.sync.dma_start(zT[:], zdram[:].rearrange("one (t b) -> (one b) t", b=BL))
    lnz0 = endp.tile([BL, 1], F32)
    nc.scalar.activation(lnz0[:], zT[:], AF.Ln, scale=2.0 ** -12)
    lnz = endp.tile([BL, 1], F32)
    nc.vector.tensor_scalar_add(lnz[:], lnz0[:], 12.0 * float(np.log(2.0)))

    # forward-side answer
    wT = endp.tile([BL, HALF], F32)
    nc.sync.dma_start(wT[:], wdram[:].rearrange("one (t b) -> (one b) t", b=BL))
    wm = endp.tile([BL, HALF], F32)
    nc.vector.tensor_tensor(out=wm[:], in0=wT[:], in1=selw_sb[:], op=ALU.mult)
    wsel = endp.tile([BL, 1], F32)
    nc.vector.tensor_reduce(wsel[:], wm[:], axis=mybir.AxisListType.X, op=ALU.add)
    wg = endp.tile([BL, 1], F32)
    nc.vector.tensor_scalar_max(wg[:], wsel[:], 1e-37)
    lnw0 = endp.tile([BL, 1], F32)
    nc.scalar.activation(lnw0[:], wg[:], AF.Ln)
    lnw = endp.tile([BL, 1], F32)
    nc.vector.tensor_scalar_add(lnw[:], lnw0[:], 29.0 * float(np.log(2.0)))

    # renorm bookkeeping: prefF = cumsum_k ln rF; cF* via selk; totals
    rFT = endp.tile([BL, 16], F32)
    nc.sync.dma_start(rFT[:], rfdram[:].rearrange("one (k b) -> (one b) k", b=BL))
    lnrf0 = endp.tile([BL, 16], F32)
    nc.scalar.activation(lnrf0[:], rFT[:], AF.Ln, scale=2.0 ** 24)
    lnrf = endp.tile([BL, 16], F32)
    nc.vector.tensor_scalar_add(lnrf[:], lnrf0[:], -24.0 * float(np.log(2.0)))
    preff = endp.tile([BL, 16], F32)
    if os.environ.get("K_NO_SCAN"):
        nc.vector.tensor_copy(preff[:, 0:1], lnrf[:, 0:1])
        for k in range(1, 16):
            nc.vector.tensor_tensor(out=preff[:, k:k + 1], in0=preff[:, k - 1:k],
                                    in1=lnrf[:, k:k + 1], op=ALU.add)
    else:
        nc.vector.tensor_tensor_scan(out=preff[:], data0=lnrf[:], data1=zeros16[:],
                                     initial=0.0, op0=ALU.add, op1=ALU.add)
    cfm = endp.tile([BL, 16], F32)
    nc.vector.tensor_tensor(out=cfm[:], in0=preff[:], in1=selk_sb[:], op=ALU.mult)
    cfsel = endp.tile([BL, 1], F32)
    nc.vector.tensor_reduce(cfsel[:], cfm[:], axis=mybir.AxisListType.X, op=ALU.add)

    rBT = endp.tile([BL, 16], F32)
    nc.sync.dma_start(rBT[:], rbdram[:].rearrange("one (k b) -> (one b) k", b=BL))
    lnrb0 = endp.tile([BL, 16], F32)
    nc.scalar.activation(lnrb0[:], rBT[:], AF.Ln, scale=2.0 ** 24)
    lnrb = endp.tile([BL, 16], F32)
    nc.vector.tensor_scalar_add(lnrb[:], lnrb0[:], -24.0 * float(np.log(2.0)))
    cbm = endp.tile([BL, 16], F32)
    nc.vector.tensor_tensor(out=cbm[:], in0=lnrb[:], in1=sbm_sb[:], op=ALU.mult)
    cbsum = endp.tile([BL, 1], F32)
    nc.vector.tensor_reduce(cbsum[:], cbm[:], axis=mybir.AxisListType.X, op=ALU.add)

    # partF = lnw - cfsel ; partB = lnz - preff[:,15] - cbsum
    partf = endp.tile([BL, 1], F32)
    nc.vector.tensor_tensor(out=partf[:], in0=lnw[:], in1=cfsel[:], op=ALU.subtract)
    pb1 = endp.tile([BL, 1], F32)
    nc.vector.tensor_tensor(out=pb1[:], in0=lnz[:], in1=preff[:, 15:16], op=ALU.subtract)
    partb = endp.tile([BL, 1], F32)
    nc.vector.tensor_tensor(out=partb[:], in0=pb1[:], in1=cbsum[:], op=ALU.subtract)

    t1 = endp.tile([BL, 1], F32)
    nc.vector.tensor_tensor(out=t1[:], in0=partf[:], in1=selfb_sb[:, 0:1], op=ALU.mult)
    t2 = endp.tile([BL, 1], F32)
    nc.vector.tensor_tensor(out=t2[:], in0=partb[:], in1=selfb_sb[:, 1:2], op=ALU.mult)
    part = endp.tile([BL, 1], F32)
    nc.vector.tensor_tensor(out=part[:], in0=t1[:], in1=t2[:], op=ALU.add)

    if dbg is not None:
        dbgt = endp.tile([BL, 8], F32)
        nc.vector.tensor_copy(dbgt[:, 0:1], lnz[:])
        nc.vector.tensor_copy(dbgt[:, 1:2], preff[:, 15:16])
        nc.vector.tensor_copy(dbgt[:, 2:3], cbsum[:])
        nc.vector.tensor_copy(dbgt[:, 3:4], lnw[:])
        nc.vector.tensor_copy(dbgt[:, 4:5], cfsel[:])
        nc.vector.tensor_copy(dbgt[:, 5:6], pair_ps[:])
        nc.vector.tensor_copy(dbgt[:, 6:7], emitsum[:])
        nc.vector.tensor_copy(dbgt[:, 7:8], wsel[:])
        nc.sync.dma_start(dbg[:], dbgt[:])
        dbgt2 = endp.tile([BL, 48], F32)
        nc.vector.tensor_copy(dbgt2[:, 0:16], rBT[:])
        nc.vector.tensor_copy(dbgt2[:, 16:32], lnrb[:])
        nc.vector.tensor_copy(dbgt2[:, 32:48], rFT[:])
        nc.sync.dma_start(dbg2[:], dbgt2[:])

    # nll = part - pairsum - emitsum
    n1 = endp.tile([BL, 1], F32)
    nc.vector.tensor_tensor(out=n1[:], in0=part[:], in1=pair_ps[:], op=ALU.subtract)
    nll = endp.tile([BL, 1], F32)
    nc.vector.tensor_tensor(out=nll[:], in0=n1[:], in1=emitsum[:], op=ALU.subtract)
    nc.sync.dma_start(out[:], nll[:])


# ---------------- host side ----------------

def _host_prep_core(feats_c, tags_c, lengths_c):
    """All host work is layout transform + integer index/mask/count prep."""
    L = lengths_c.astype(np.int64)
    tg = tags_c.astype(np.int64)

    ftg = np.empty(FTG_N, np.float32)
    ftg[:T * M * BL] = np.ascontiguousarray(feats_c.transpose(2, 1, 0)).ravel()
    ftg[T * M * BL:] = 0.0

    bidx = np.arange(BL)
    tidx = np.arange(M)
    mask = tidx[None, :] < L[:, None]
    # emission gather done host-side (selection only, no arithmetic): HW
    # indirect DMA only supports one contiguous run per partition, not
    # per-element gathers
    emitv = np.where(mask, feats_c[bidx[:, None], tidx[None, :], tg], 0.0)
    emitv = np.ascontiguousarray(emitv, np.float32)

    inj = np.zeros((129, BL), np.float32)
    big = L >= 128
    inj[L[big] - 1 - 127, bidx[big]] = 1.0

    selw = np.zeros((BL, HALF), np.float32)
    small = L <= 127
    selw[bidx[small], L[small] - 1] = 1.0

    selk = np.zeros((BL, 16), np.float32)
    K = np.minimum((L - 1) // 8, 15)
    pick = small & (K >= 1)
    selk[bidx[pick], K[pick] - 1] = 1.0

    sbm = np.zeros((BL, 16), np.float32)
    s_k = 4 + 8 * np.arange(16)
    # renorm k's factor is applied to column b iff b was injected at or before
    # that renorm's step (injection at tau=L-1, renorm reads state at 256-s_k)
    sbm[:, :] = (L[:, None] >= 256 - s_k[None, :]).astype(np.float32)

    selfb = np.stack([small, big], axis=1).astype(np.float32)

    cfull = np.zeros((T * T, BL), np.float32)
    prev = tg[:, :-1]
    nxt = tg[:, 1:]
    pmask = (tidx[1:][None, :] < L[:, None])
    pidx = (prev * T + nxt)
    np.add.at(cfull, (pidx[pmask], np.broadcast_to(bidx[:, None], pidx.shape)[pmask]), 1.0)
    np.add.at(cfull, (ROOT * T + tg[:, 0], bidx), 1.0)
    last = tg[bidx, L - 1]
    np.add.at(cfull, (last * T + END, bidx), 1.0)
    cmat = np.ascontiguousarray(
        cfull.reshape(18, 128, BL).transpose(1, 0, 2)).reshape(128, 18 * BL)

    return {
        "ftg": ftg,
        "emitv": emitv,
        "inj": inj.reshape(1, 129 * BL),
        "selw": selw,
        "selk": selk,
        "sbm": sbm,
        "selfb": selfb,
        "cmat": cmat,
    }


def kernel(feats, tags, lengths, log_transitions):
    global _PROGRAM
    feats = np.asarray(feats, np.float32)
    tags = np.asarray(tags)
    lengths = np.asarray(lengths)
    lt = np.asarray(log_transitions, np.float32)
    ident = np.eye(T, dtype=np.float32)

    in_maps = []
    for c in range(NC):
        sl = slice(c * BL, (c + 1) * BL)
        m = _host_prep_core(feats[sl], tags[sl], lengths[sl])
        m["lt"] = lt
        m["ident"] = ident
        in_maps.append(m)

    if _PROGRAM is None:
        _PROGRAM = _build_program()

    res = run_bass_kernel_spmd(_PROGRAM, in_maps, core_ids=list(range(NC)))
    return np.concatenate([r["out"].reshape(BL) for r in res.results])


if __name__ == "__main__":
    rng = np.random.default_rng(0)
    feats = rng.standard_normal((B, M, T)).astype(np.float32)
    tags = rng.integers(0, ROOT, (B, M)).astype(np.int32)
    lengths = rng.integers(1, M + 1, (B,)).astype(np.int32)
    std = (2.0 / (T + T)) ** 0.5
    lt = (rng.standard_normal((T, T)) * std).astype(np.float32)
    lt[:, ROOT] = -10000.0
    lt[END, :] = -10000.0
    out = kernel(feats, tags, lengths, lt)
    print(out[:8], out.shape, out.dtype)



# revision 6
# speedup vs baseline: 1.3391x; 1.3391x over previous
"""Chain-CRF negative log-likelihood on 8 Trainium2 NeuronCores (Bass/Tile).

Strategy (pure data parallelism, batch 512 -> 64 per core, v2 latency-optimized):
  Scaled-exp forward algorithm, meet-in-the-middle: a forward chain (alpha,
  t = 0..127) and a backward chain (beta, t = 255..127) run concurrently as
  two independent serial dependency chains; the per-step round trip is
  PE matmul -> GPSIMD (Pool) elementwise multiply -> PE.

  Layout: the 64 sequences are split into two halves of 32 stacked on the
  partition axis, so each chain's state is [96, 32] (rows 0-47 = half 0,
  48-95 = half 1) and each step is ONE block-diagonal matmul (free dim 32)
  plus ONE Pool multiply. Variable lengths:
    - forward: the matmul's aux output rows 96/97 give w_{t-1} = Tend.p for
      free; sequences with L <= 127 read ln w_{L-1} at the end.
    - backward: injection of sequence b's beta chain at t = L_b - 1 rides in
      the matmul as state rows 96/97 (a 0/1 indicator written ahead of time
      by DVE) against a Tend row of the stationary matrix - no extra matmul.
  Stability: every transition entry carries exp(-c) (c = 4.3, folded into
  the stationary matrices at setup by ScalarE's activation bias), which
  cancels the mean per-step growth; state magnitudes then stay within
  e^{+-25} over 128 steps, so NO renormalization is needed. The exact
  compensation is the deterministic + c*L term added at the end.

  Gold path score: host gathers the emission/transition VALUES (selection
  only - HW indirect DMA cannot do per-element gathers); the device sums
  them in one reduction. All floating-point arithmetic happens on device;
  the host does layout transforms and integer index/mask preprocessing.
"""

import os

os.environ.setdefault("NEURON_CC_FLAGS", "")

import numpy as np
from contextlib import ExitStack

import concourse.bass as bass
import concourse.tile as tile
from concourse import bacc, mybir
from concourse.bass_utils import run_bass_kernel_spmd

# ---- problem constants (hardcoded per contract) ----
B = 512
M = 256
T = 48          # n_tags
ROOT = 46
END = 47
NC = 8
BL = B // NC    # 64 sequences per core
H = 32          # half-batch (column) width per chain
NEG = -10000.0
C = 4.3         # per-step scale; exp(-C) folded into transitions

F32 = mybir.dt.float32
AF = mybir.ActivationFunctionType
ALU = mybir.AluOpType

_PROGRAM = None


def _build_program():
    nc = bacc.Bacc(
        "TRN2",
        target_bir_lowering=False,
        debug=False,
        enable_asserts=False,
        num_devices=NC,
    )

    efraw = nc.dram_tensor("efraw", [96, 128 * H], F32, kind="ExternalInput").ap()
    ebraw = nc.dram_tensor("ebraw", [96, 128 * H], F32, kind="ExternalInput").ap()
    talogf = nc.dram_tensor("talogf", [96, 98], F32, kind="ExternalInput").ap()
    talogb = nc.dram_tensor("talogb", [98, 96], F32, kind="ExternalInput").ap()
    tendlog2 = nc.dram_tensor("tendlog2", [2, 96], F32, kind="ExternalInput").ap()
    rootlog2 = nc.dram_tensor("rootlog2", [96, 1], F32, kind="ExternalInput").ap()
    halfsel = nc.dram_tensor("halfsel", [96, 2], F32, kind="ExternalInput").ap()
    injs = nc.dram_tensor("injs", [2, 129 * H], F32, kind="ExternalInput").ap()
    goldv = nc.dram_tensor("goldv", [BL, 516], F32, kind="ExternalInput").ap()
    selw = nc.dram_tensor("selw", [BL, 128], F32, kind="ExternalInput").ap()
    selfb = nc.dram_tensor("selfb", [BL, 2], F32, kind="ExternalInput").ap()
    lfl = nc.dram_tensor("lfl", [BL, 1], F32, kind="ExternalInput").ap()
    out = nc.dram_tensor("out", [BL, 1], F32, kind="ExternalOutput").ap()

    with tile.TileContext(nc) as tc, ExitStack() as ctx:
        _emit_body(ctx, tc, efraw, ebraw, talogf, talogb, tendlog2, rootlog2,
                   halfsel, injs, goldv, selw, selfb, lfl, out)
    nc.finalize()
    return nc


def _emit_body(ctx, tc, efraw, ebraw, talogf, talogb, tendlog2, rootlog2,
               halfsel, injs, goldv, selw, selfb, lfl, out):
    nc = tc.nc

    const = ctx.enter_context(tc.tile_pool(name="const", bufs=1))
    raws = ctx.enter_context(tc.tile_pool(name="raws", bufs=4))
    slab = ctx.enter_context(tc.tile_pool(name="slab", bufs=1))
    sf_pool = ctx.enter_context(tc.tile_pool(name="sf", bufs=4))
    sb_pool = ctx.enter_context(tc.tile_pool(name="sb", bufs=4))
    wfl = ctx.enter_context(tc.tile_pool(name="wfl", bufs=2))
    endp = ctx.enter_context(tc.tile_pool(name="endp", bufs=2))
    pf_pool = ctx.enter_context(tc.tile_pool(name="pf", bufs=2, space="PSUM"))
    pb_pool = ctx.enter_context(tc.tile_pool(name="pb", bufs=2, space="PSUM"))
    ip_pool = ctx.enter_context(tc.tile_pool(name="ip", bufs=2, space="PSUM"))
    dram = ctx.enter_context(tc.tile_pool(name="dram", bufs=1, space="DRAM"))

    # ---------------- constants ----------------
    biasc = const.tile([98, 1], F32)
    nc.gpsimd.memset(biasc[:], -C)

    tafl = const.tile([96, 98], F32)
    nc.sync.dma_start(tafl[:], talogf[:])
    TAF = const.tile([96, 98], F32)
    nc.scalar.activation(TAF[:], tafl[:], AF.Exp, bias=biasc[0:96, 0:1])

    tabl = const.tile([98, 96], F32)
    nc.sync.dma_start(tabl[:], talogb[:])
    TAB = const.tile([98, 96], F32)
    nc.scalar.activation(TAB[:], tabl[:], AF.Exp, bias=biasc[0:98, 0:1])

    tel = const.tile([2, 96], F32)
    nc.sync.dma_start(tel[:], tendlog2[:])
    TENDC = const.tile([2, 96], F32)
    nc.scalar.activation(TENDC[:], tel[:], AF.Exp, bias=biasc[0:2, 0:1])

    rl = const.tile([96, 1], F32)
    nc.sync.dma_start(rl[:], rootlog2[:])
    ROOTE = const.tile([96, 1], F32)
    nc.scalar.activation(ROOTE[:], rl[:], AF.Exp)

    HS = const.tile([96, 2], F32)
    nc.sync.dma_start(HS[:], halfsel[:])
    INJ = const.tile([2, 129 * H], F32)
    nc.sync.dma_start(INJ[:], injs[:])
    GOLDV = const.tile([BL, 516], F32)
    nc.sync.dma_start(GOLDV[:], goldv[:])
    SELW = const.tile([BL, 128], F32)
    nc.sync.dma_start(SELW[:], selw[:])
    SELFB = const.tile([BL, 2], F32)
    nc.sync.dma_start(SELFB[:], selfb[:])
    LFL = const.tile([BL, 1], F32)
    nc.sync.dma_start(LFL[:], lfl[:])

    # gold sum (independent of the scan)
    gsum = const.tile([BL, 1], F32)
    nc.vector.tensor_reduce(gsum[:], GOLDV[:], axis=mybir.AxisListType.X, op=ALU.add)

    # dram scratch for w history / z transposition
    wdram = dram.tile([2, 128 * H], F32, tag="wdram")
    zdram = dram.tile([1, BL], F32, tag="zdram")
    # w col 127 is never produced by the scan; keep it defined
    zrow32 = const.tile([2, H], F32)
    nc.gpsimd.memset(zrow32[:], 0.0)
    nc.sync.dma_start(wdram[:, 127 * H:128 * H], zrow32[:])

    # ---------------- exp(feats) slabs ----------------
    EF = slab.tile([96, 128 * H], F32)
    EB = slab.tile([96, 128 * H], F32)
    pieces = [("f", 0, 64), ("b", 0, 64), ("f", 64, 448), ("b", 64, 448)]
    for k in range(7):
        pieces += [("f", 512 * (k + 1), 512), ("b", 512 * (k + 1), 512)]
    for which, off, ln in pieces:
        raw = raws.tile([96, 512], F32, tag="raw")
        src = efraw if which == "f" else ebraw
        dst = EF if which == "f" else EB
        nc.sync.dma_start(raw[:, 0:ln], src[:, off:off + ln])
        nc.scalar.activation(dst[:, off:off + ln], raw[:, 0:ln], AF.Exp)

    # ---------------- init ----------------
    sfp = sf_pool.tile([96, H], F32, tag="sf")
    nc.vector.tensor_scalar_mul(sfp[:], EF[:, 0:H], ROOTE[:, 0:1])

    ipt = ip_pool.tile([96, H], F32, space="PSUM", tag="ip")
    nc.tensor.matmul(out=ipt[:], lhsT=TENDC[:], rhs=INJ[:, 0:H], start=True, stop=True)
    sbp = sb_pool.tile([98, H], F32, tag="sb")
    nc.vector.tensor_tensor(out=sbp[0:96, :], in0=ipt[:], in1=EB[:, 0:H], op=ALU.mult)
    nc.vector.tensor_copy(sbp[96:98, :], INJ[:, H:2 * H])

    # ---------------- the scan: 128 iterations, two chains ----------------
    pf = None
    pb = None
    sf127 = None
    beta = None
    for i in range(1, 129):
        col = (i - 1) % 16
        blk = (i - 1) // 16
        if i <= 127:
            if col == 0:
                pf = pf_pool.tile([98, 512], F32, space="PSUM")
            nc.tensor.matmul(out=pf[:, col * H:(col + 1) * H], lhsT=TAF[:],
                             rhs=sfp[:], start=True, stop=True)
            sfn = sf_pool.tile([96, H], F32, tag="sf")
            nc.vector.tensor_tensor(out=sfn[:], in0=pf[0:96, col * H:(col + 1) * H],
                                    in1=EF[:, i * H:(i + 1) * H], op=ALU.mult)
            if col == 15 or i == 127:
                ncols = (col + 1) * H
                wst = wfl.tile([2, 512], F32, tag="wst")
                nc.scalar.activation(wst[:, 0:ncols], pf[96:98, 0:ncols], AF.Copy)
                nc.sync.dma_start(wdram[:, blk * 512:blk * 512 + ncols],
                                  wst[:, 0:ncols])
            sfp = sfn
            if i == 127:
                sf127 = sfn
        if col == 0:
            pb = pb_pool.tile([96, 512], F32, space="PSUM")
        nc.tensor.matmul(out=pb[:, col * H:(col + 1) * H], lhsT=TAB[:],
                         rhs=sbp[:], start=True, stop=True)
        if i <= 127:
            sbn = sb_pool.tile([98, H], F32, tag="sb")
            nc.vector.tensor_tensor(out=sbn[0:96, :], in0=pb[0:96, col * H:(col + 1) * H],
                                    in1=EB[:, i * H:(i + 1) * H], op=ALU.mult)
            nc.vector.tensor_copy(sbn[96:98, :], INJ[:, (i + 1) * H:(i + 2) * H])
            sbp = sbn
        else:
            beta = pb[0:96, col * H:(col + 1) * H]

    # ---------------- combine & finalize ----------------
    zz = endp.tile([96, H], F32)
    nc.vector.tensor_tensor(out=zz[:], in0=sf127[:], in1=beta, op=ALU.mult)
    zp = ip_pool.tile([2, H], F32, space="PSUM", tag="zp")
    nc.tensor.matmul(out=zp[:], lhsT=HS[:], rhs=zz[:], start=True, stop=True)
    zrow = endp.tile([2, H], F32)
    nc.vector.tensor_scalar_max(zrow[:], zp[:], 1e-37)
    nc.sync.dma_start(zdram[:].rearrange("one (h m) -> (one h) m", h=2), zrow[:])
    zT = endp.tile([BL, 1], F32)
    nc.sync.dma_start(zT[:], zdram[:].rearrange("one (b o) -> (one b) o", o=1))
    lnz = endp.tile([BL, 1], F32)
    nc.scalar.activation(lnz[:], zT[:], AF.Ln)

    wT = endp.tile([BL, 128], F32)
    nc.sync.dma_start(wT[0:H, :], wdram[0:1, :].rearrange("one (t m) -> (one m) t", m=H))
    nc.sync.dma_start(wT[H:2 * H, :], wdram[1:2, :].rearrange("one (t m) -> (one m) t", m=H))
    wm = endp.tile([BL, 128], F32)
    nc.vector.tensor_tensor(out=wm[:], in0=wT[:], in1=SELW[:], op=ALU.mult)
    wsel = endp.tile([BL, 1], F32)
    nc.vector.tensor_reduce(wsel[:], wm[:], axis=mybir.AxisListType.X, op=ALU.add)
    wg = endp.tile([BL, 1], F32)
    nc.vector.tensor_scalar_max(wg[:], wsel[:], 1e-37)
    lnw = endp.tile([BL, 1], F32)
    nc.scalar.activation(lnw[:], wg[:], AF.Ln)

    t1 = endp.tile([BL, 1], F32)
    nc.vector.tensor_tensor(out=t1[:], in0=lnw[:], in1=SELFB[:, 0:1], op=ALU.mult)
    t2 = endp.tile([BL, 1], F32)
    nc.vector.tensor_tensor(out=t2[:], in0=lnz[:], in1=SELFB[:, 1:2], op=ALU.mult)
    p0 = endp.tile([BL, 1], F32)
    nc.vector.tensor_tensor(out=p0[:], in0=t1[:], in1=t2[:], op=ALU.add)
    # part = C*L + p0 ; nll = part - gold
    part = endp.tile([BL, 1], F32)
    nc.vector.scalar_tensor_tensor(out=part[:], in0=LFL[:], scalar=C, in1=p0[:],
                                   op0=ALU.mult, op1=ALU.add)
    nll = endp.tile([BL, 1], F32)
    nc.vector.tensor_tensor(out=nll[:], in0=part[:], in1=gsum[:], op=ALU.subtract)
    nc.sync.dma_start(out[:], nll[:])


# ---------------- host side ----------------

def _host_prep_core(feats_c, tags_c, lengths_c, lt):
    """All host work is layout transform + integer index/mask prep + value
    gathers (selection); every FP arithmetic op runs on device."""
    L = lengths_c.astype(np.int64)
    tg = tags_c.astype(np.int64)
    f32 = np.float32

    f_t = np.ascontiguousarray(feats_c.transpose(2, 1, 0))  # [48, 256, 64]
    ef = np.empty((96, 128 * H), f32)
    ef[0:48] = f_t[:, 0:128, 0:H].reshape(48, 128 * H)
    ef[48:96] = f_t[:, 0:128, H:2 * H].reshape(48, 128 * H)
    fb = f_t[:, 255:127:-1, :]                              # tau = 255..128
    eb = np.empty((96, 128 * H), f32)
    eb[0:48] = fb[:, :, 0:H].reshape(48, 128 * H)
    eb[48:96] = fb[:, :, H:2 * H].reshape(48, 128 * H)

    # stationary matrices (log space; NEG -> exp 0)
    talogf = np.full((96, 98), NEG, f32)
    talogf[0:48, 0:48] = lt
    talogf[48:96, 48:96] = lt
    talogf[0:48, 96] = lt[:, END]
    talogf[48:96, 97] = lt[:, END]
    talogb = np.full((98, 96), NEG, f32)
    talogb[0:48, 0:48] = lt.T
    talogb[48:96, 48:96] = lt.T
    talogb[96, 0:48] = lt[:, END]
    talogb[97, 48:96] = lt[:, END]
    tendlog2 = np.full((2, 96), NEG, f32)
    tendlog2[0, 0:48] = lt[:, END]
    tendlog2[1, 48:96] = lt[:, END]
    rootlog2 = np.concatenate([lt[ROOT, :], lt[ROOT, :]]).astype(f32)[:, None]
    halfsel = np.zeros((96, 2), f32)
    halfsel[0:48, 0] = 1.0
    halfsel[48:96, 1] = 1.0

    # injection indicators: block i <-> L == 256 - i, i = 0..128
    Lh = L.reshape(2, H)                                    # [2, 32]
    ivals = 256 - np.arange(129)
    inj = (Lh[:, None, :] == ivals[None, :, None]).astype(f32)  # [2, 129, 32]
    inj = inj.reshape(2, 129 * H)

    # gold values (host-gathered, device-summed)
    bidx = np.arange(BL)
    tidx = np.arange(M)
    mask = tidx[None, :] < L[:, None]
    emitv = np.where(mask, feats_c[bidx[:, None], tidx[None, :], tg], 0.0)
    pmask = tidx[1:][None, :] < L[:, None]
    pairv = np.where(pmask, lt[tg[:, :-1], tg[:, 1:]], 0.0)
    goldv = np.zeros((BL, 516), f32)
    goldv[:, 0:256] = emitv
    goldv[:, 256:511] = pairv
    goldv[:, 511] = lt[ROOT, tg[:, 0]]
    goldv[:, 512] = lt[tg[bidx, L - 1], END]

    selw = np.zeros((BL, 128), f32)
    small = L <= 127
    selw[bidx[small], L[small] - 1] = 1.0
    selfb = np.stack([small, ~small], axis=1).astype(f32)
    lfl = L.astype(f32)[:, None]

    return {
        "efraw": ef,
        "ebraw": eb,
        "talogf": talogf,
        "talogb": talogb,
        "tendlog2": tendlog2,
        "rootlog2": rootlog2,
        "halfsel": halfsel,
        "injs": inj,
        "goldv": goldv,
        "selw": selw,
        "selfb": selfb,
        "lfl": lfl,
    }


def kernel(feats, tags, lengths, log_transitions):
    global _PROGRAM
    feats = np.asarray(feats, np.float32)
    tags = np.asarray(tags)
    lengths = np.asarray(lengths)
    lt = np.asarray(log_transitions, np.float32)

    in_maps = []
    for c in range(NC):
        sl = slice(c * BL, (c + 1) * BL)
        in_maps.append(_host_prep_core(feats[sl], tags[sl], lengths[sl], lt))

    if _PROGRAM is None:
        _PROGRAM = _build_program()

    res = run_bass_kernel_spmd(_PROGRAM, in_maps, core_ids=list(range(NC)))
    return np.concatenate([r["out"].reshape(BL) for r in res.results])


if __name__ == "__main__":
    rng = np.random.default_rng(0)
    feats = rng.standard_normal((B, M, T)).astype(np.float32)
    tags = rng.integers(0, ROOT, (B, M)).astype(np.int32)
    lengths = rng.integers(1, M + 1, (B,)).astype(np.int32)
    std = (2.0 / (T + T)) ** 0.5
    lt = (rng.standard_normal((T, T)) * std).astype(np.float32)
    lt[:, ROOT] = NEG
    lt[END, :] = NEG
    out = kernel(feats, tags, lengths, lt)
    print(out[:8], out.shape, out.dtype)


# revision 11
# speedup vs baseline: 1.4449x; 1.0791x over previous
"""Chain-CRF negative log-likelihood on 8 Trainium2 NeuronCores (Bass/Tile).

Strategy (pure data parallelism, batch 512 -> 64 per core, v2 latency-optimized):
  Scaled-exp forward algorithm, meet-in-the-middle: a forward chain (alpha,
  t = 0..127) and a backward chain (beta, t = 255..127) run concurrently as
  two independent serial dependency chains; the per-step round trip is
  PE matmul -> GPSIMD (Pool) elementwise multiply -> PE.

  Layout: the 64 sequences are split into two halves of 32 stacked on the
  partition axis, so each chain's state is [96, 32] (rows 0-47 = half 0,
  48-95 = half 1) and each step is ONE block-diagonal matmul (free dim 32)
  plus ONE Pool multiply. Variable lengths:
    - forward: the matmul's aux output rows 96/97 give w_{t-1} = Tend.p for
      free; sequences with L <= 127 read ln w_{L-1} at the end.
    - backward: injection of sequence b's beta chain at t = L_b - 1 rides in
      the matmul as state rows 96/97 (a 0/1 indicator written ahead of time
      by DVE) against a Tend row of the stationary matrix - no extra matmul.
  Stability: every transition entry carries exp(-c) (c = 4.3, folded into
  the stationary matrices at setup by ScalarE's activation bias), which
  cancels the mean per-step growth; state magnitudes then stay within
  e^{+-25} over 128 steps, so NO renormalization is needed. The exact
  compensation is the deterministic + c*L term added at the end.

  Gold path score: host gathers the emission/transition VALUES (selection
  only - HW indirect DMA cannot do per-element gathers); the device sums
  them in one reduction. All floating-point arithmetic happens on device;
  the host does layout transforms and integer index/mask preprocessing.
"""

import os

os.environ.setdefault("NEURON_CC_FLAGS", "")

import numpy as np
from contextlib import ExitStack

import concourse.bass as bass
import concourse.tile as tile
from concourse import bacc, mybir
from concourse.bass_utils import run_bass_kernel_spmd

# ---- problem constants (hardcoded per contract) ----
B = 512
M = 256
T = 48          # n_tags
ROOT = 46
END = 47
NC = 8
BL = B // NC    # 64 sequences per core
H = 32          # half-batch (column) width per chain
NEG = -10000.0
C = 4.3         # per-step scale; exp(-C) folded into transitions

F32 = mybir.dt.float32
AF = mybir.ActivationFunctionType
ALU = mybir.AluOpType

_PROGRAM = None


def _build_program():
    nc = bacc.Bacc(
        "TRN2",
        target_bir_lowering=False,
        debug=False,
        enable_asserts=False,
        num_devices=NC,
    )

    efraw = nc.dram_tensor("efraw", [96, 128 * H], F32, kind="ExternalInput").ap()
    ebraw = nc.dram_tensor("ebraw", [96, 128 * H], F32, kind="ExternalInput").ap()
    talogf = nc.dram_tensor("talogf", [96, 98], F32, kind="ExternalInput").ap()
    talogb = nc.dram_tensor("talogb", [98, 96], F32, kind="ExternalInput").ap()
    tendlog2 = nc.dram_tensor("tendlog2", [2, 96], F32, kind="ExternalInput").ap()
    rootlog2 = nc.dram_tensor("rootlog2", [96, 1], F32, kind="ExternalInput").ap()
    halfsel = nc.dram_tensor("halfsel", [96, 2], F32, kind="ExternalInput").ap()
    injs = nc.dram_tensor("injs", [2, 129 * H], F32, kind="ExternalInput").ap()
    goldv = nc.dram_tensor("goldv", [BL, 516], F32, kind="ExternalInput").ap()
    selw = nc.dram_tensor("selw", [BL, 128], F32, kind="ExternalInput").ap()
    selfb = nc.dram_tensor("selfb", [BL, 2], F32, kind="ExternalInput").ap()
    lfl = nc.dram_tensor("lfl", [BL, 1], F32, kind="ExternalInput").ap()
    out = nc.dram_tensor("out", [BL, 1], F32, kind="ExternalOutput").ap()

    with tile.TileContext(nc) as tc, ExitStack() as ctx:
        _emit_body(ctx, tc, efraw, ebraw, talogf, talogb, tendlog2, rootlog2,
                   halfsel, injs, goldv, selw, selfb, lfl, out)
    nc.finalize()
    return nc


def _emit_body(ctx, tc, efraw, ebraw, talogf, talogb, tendlog2, rootlog2,
               halfsel, injs, goldv, selw, selfb, lfl, out):
    nc = tc.nc

    const = ctx.enter_context(tc.tile_pool(name="const", bufs=1))
    raws = ctx.enter_context(tc.tile_pool(name="raws", bufs=4))
    slab = ctx.enter_context(tc.tile_pool(name="slab", bufs=1))
    sf_pool = ctx.enter_context(tc.tile_pool(name="sf", bufs=4))
    sb_pool = ctx.enter_context(tc.tile_pool(name="sb", bufs=4))
    wfl = ctx.enter_context(tc.tile_pool(name="wfl", bufs=2))
    endp = ctx.enter_context(tc.tile_pool(name="endp", bufs=2))
    pf_pool = ctx.enter_context(tc.tile_pool(name="pf", bufs=2, space="PSUM"))
    pb_pool = ctx.enter_context(tc.tile_pool(name="pb", bufs=2, space="PSUM"))
    ip_pool = ctx.enter_context(tc.tile_pool(name="ip", bufs=2, space="PSUM"))
    dram = ctx.enter_context(tc.tile_pool(name="dram", bufs=1, space="DRAM"))

    # ---------------- scan-critical constants first ----------------
    biasc = const.tile([98, 1], F32)
    nc.gpsimd.memset(biasc[:], -C)

    EF = slab.tile([96, 128 * H], F32)
    EB = slab.tile([96, 128 * H], F32)

    def load_piece(which, off, ln):
        raw = raws.tile([96, 512], F32, tag="raw")
        src = efraw if which == "f" else ebraw
        dst = EF if which == "f" else EB
        nc.sync.dma_start(raw[:, 0:ln], src[:, off:off + ln])
        nc.scalar.activation(dst[:, off:off + ln], raw[:, 0:ln], AF.Exp)

    load_piece("f", 0, 64)
    load_piece("b", 0, 64)

    tafl = const.tile([96, 98], F32)
    nc.sync.dma_start(tafl[:], talogf[:])
    TAF = const.tile([96, 98], F32)
    nc.scalar.activation(TAF[:], tafl[:], AF.Exp, bias=biasc[0:96, 0:1])

    tabl = const.tile([98, 96], F32)
    nc.sync.dma_start(tabl[:], talogb[:])
    TAB = const.tile([98, 96], F32)
    nc.scalar.activation(TAB[:], tabl[:], AF.Exp, bias=biasc[0:98, 0:1])

    tel = const.tile([2, 96], F32)
    nc.sync.dma_start(tel[:], tendlog2[:])
    TENDC = const.tile([2, 96], F32)
    nc.scalar.activation(TENDC[:], tel[:], AF.Exp, bias=biasc[0:2, 0:1])

    rl = const.tile([96, 1], F32)
    nc.sync.dma_start(rl[:], rootlog2[:])
    ROOTE = const.tile([96, 1], F32)
    nc.scalar.activation(ROOTE[:], rl[:], AF.Exp)

    INJ = const.tile([2, 129 * H], F32)
    nc.sync.dma_start(INJ[:], injs[:])

    # ---------------- exp(feats) slabs (rest) ----------------
    pieces = [("f", 64, 448), ("b", 64, 448)]
    for k in range(7):
        pieces += [("f", 512 * (k + 1), 512), ("b", 512 * (k + 1), 512)]
    for which, off, ln in pieces:
        load_piece(which, off, ln)

    # ---------------- non-critical constants ----------------
    HS = const.tile([96, 2], F32)
    nc.sync.dma_start(HS[:], halfsel[:])
    GOLDV = const.tile([BL, 516], F32)
    nc.sync.dma_start(GOLDV[:], goldv[:])
    SELW = const.tile([BL, 128], F32)
    nc.sync.dma_start(SELW[:], selw[:])
    SELFB = const.tile([BL, 2], F32)
    nc.sync.dma_start(SELFB[:], selfb[:])
    LFL = const.tile([BL, 1], F32)
    nc.sync.dma_start(LFL[:], lfl[:])

    # gold sum (independent of the scan)
    gsum = const.tile([BL, 1], F32)
    nc.vector.tensor_reduce(gsum[:], GOLDV[:], axis=mybir.AxisListType.X, op=ALU.add)

    # dram scratch: per-chunk w tiles (b-major so end loads are contiguous)
    wdram = [dram.tile([1, 1024], F32, name=f"wd{k}", tag=f"wd{k}") for k in range(8)]
    zdram = dram.tile([1, BL], F32, tag="zdram")
    # w col 127 is never produced by the scan; keep it defined
    zrow32 = const.tile([BL, 16], F32)
    nc.gpsimd.memset(zrow32[:], 0.0)

    # ---------------- init ----------------
    sfp = sf_pool.tile([96, H], F32, tag="sf")
    nc.vector.tensor_scalar_mul(sfp[:], EF[:, 0:H], ROOTE[:, 0:1])

    ipt = ip_pool.tile([96, H], F32, space="PSUM", tag="ip")
    nc.tensor.matmul(out=ipt[:], lhsT=TENDC[:], rhs=INJ[:, 0:H], start=True, stop=True)
    sbp = sb_pool.tile([98, H], F32, tag="sb")
    nc.vector.tensor_tensor(out=sbp[0:96, :], in0=ipt[:], in1=EB[:, 0:H], op=ALU.mult)
    nc.vector.tensor_copy(sbp[96:98, :], INJ[:, H:2 * H])

    # ---------------- the scan: 128 iterations, two chains ----------------
    wT = const.tile([BL, 128], F32)
    pf = None
    pb = None
    sf127 = None
    beta = None
    for i in range(1, 129):
        col = (i - 1) % 16
        blk = (i - 1) // 16
        if col == 0:
            pf = pf_pool.tile([98, 512], F32, space="PSUM")
        nc.tensor.matmul(out=pf[:, col * H:(col + 1) * H], lhsT=TAF[:],
                         rhs=sfp[:], start=True, stop=True)
        if i <= 127:
            sfn = sf_pool.tile([96, H], F32, tag="sf")
            nc.vector.tensor_tensor(out=sfn[:], in0=pf[0:96, col * H:(col + 1) * H],
                                    in1=EF[:, i * H:(i + 1) * H], op=ALU.mult)
        if col == 15:
            # stage w_{16*blk..16*blk+15}: transpose-store so the end-phase
            # load is contiguous per partition, then load back immediately
            wst = wfl.tile([2, 512], F32, tag="wst")
            nc.scalar.activation(
                wst[:].rearrange("p (m c) -> p m c", m=H, c=16),
                pf[96:98, :].rearrange("p (c m) -> p m c", c=16, m=H),
                AF.Copy)
            nc.sync.dma_start(
                wdram[blk][:].rearrange("one (h x) -> (one h) x", h=2),
                wst[:])
            nc.sync.dma_start(
                wT[:, blk * 16:(blk + 1) * 16],
                wdram[blk][:].rearrange("one (b c) -> (one b) c", b=BL))
        if i <= 127:
            sfp = sfn
            if i == 127:
                sf127 = sfn
        if col == 0:
            pb = pb_pool.tile([96, 512], F32, space="PSUM")
        nc.tensor.matmul(out=pb[:, col * H:(col + 1) * H], lhsT=TAB[:],
                         rhs=sbp[:], start=True, stop=True)
        if i <= 127:
            sbn = sb_pool.tile([98, H], F32, tag="sb")
            nc.vector.tensor_tensor(out=sbn[0:96, :], in0=pb[0:96, col * H:(col + 1) * H],
                                    in1=EB[:, i * H:(i + 1) * H], op=ALU.mult)
            nc.vector.tensor_copy(sbn[96:98, :], INJ[:, (i + 1) * H:(i + 2) * H])
            sbp = sbn
        else:
            beta = pb[0:96, col * H:(col + 1) * H]

    # ---------------- combine & finalize ----------------
    zz = endp.tile([96, H], F32)
    nc.vector.tensor_tensor(out=zz[:], in0=sf127[:], in1=beta, op=ALU.mult)
    zp = ip_pool.tile([2, H], F32, space="PSUM", tag="zp")
    nc.tensor.matmul(out=zp[:], lhsT=HS[:], rhs=zz[:], start=True, stop=True)
    zrow = endp.tile([2, H], F32)
    nc.vector.tensor_scalar_max(zrow[:], zp[:], 1e-37)
    nc.sync.dma_start(zdram[:].rearrange("one (h m) -> (one h) m", h=2), zrow[:])
    zT = endp.tile([BL, 1], F32)
    nc.sync.dma_start(zT[:], zdram[:].rearrange("one (b o) -> (one b) o", o=1))
    lnz = endp.tile([BL, 1], F32)
    nc.scalar.activation(lnz[:], zT[:], AF.Ln)

    wm = endp.tile([BL, 128], F32)
    nc.vector.tensor_tensor(out=wm[:], in0=wT[:], in1=SELW[:], op=ALU.mult)
    wsel = endp.tile([BL, 1], F32)
    nc.vector.tensor_reduce(wsel[:], wm[:], axis=mybir.AxisListType.X, op=ALU.add)
    wg = endp.tile([BL, 1], F32)
    nc.vector.tensor_scalar_max(wg[:], wsel[:], 1e-37)
    lnw = endp.tile([BL, 1], F32)
    nc.scalar.activation(lnw[:], wg[:], AF.Ln)

    t1 = endp.tile([BL, 1], F32)
    nc.vector.tensor_tensor(out=t1[:], in0=lnw[:], in1=SELFB[:, 0:1], op=ALU.mult)
    t2 = endp.tile([BL, 1], F32)
    nc.vector.tensor_tensor(out=t2[:], in0=lnz[:], in1=SELFB[:, 1:2], op=ALU.mult)
    p0 = endp.tile([BL, 1], F32)
    nc.vector.tensor_tensor(out=p0[:], in0=t1[:], in1=t2[:], op=ALU.add)
    # part = C*L + p0 ; nll = part - gold
    part = endp.tile([BL, 1], F32)
    nc.vector.scalar_tensor_tensor(out=part[:], in0=LFL[:], scalar=C, in1=p0[:],
                                   op0=ALU.mult, op1=ALU.add)
    nll = endp.tile([BL, 1], F32)
    nc.vector.tensor_tensor(out=nll[:], in0=part[:], in1=gsum[:], op=ALU.subtract)
    nc.sync.dma_start(out[:], nll[:])


# ---------------- host side ----------------

def _host_prep_core(feats_c, tags_c, lengths_c, lt):
    """All host work is layout transform + integer index/mask prep + value
    gathers (selection); every FP arithmetic op runs on device."""
    L = lengths_c.astype(np.int64)
    tg = tags_c.astype(np.int64)
    f32 = np.float32

    f_t = np.ascontiguousarray(feats_c.transpose(2, 1, 0))  # [48, 256, 64]
    ef = np.empty((96, 128 * H), f32)
    ef[0:48] = f_t[:, 0:128, 0:H].reshape(48, 128 * H)
    ef[48:96] = f_t[:, 0:128, H:2 * H].reshape(48, 128 * H)
    fb = f_t[:, 255:127:-1, :]                              # tau = 255..128
    eb = np.empty((96, 128 * H), f32)
    eb[0:48] = fb[:, :, 0:H].reshape(48, 128 * H)
    eb[48:96] = fb[:, :, H:2 * H].reshape(48, 128 * H)

    # stationary matrices (log space; NEG -> exp 0)
    talogf = np.full((96, 98), NEG, f32)
    talogf[0:48, 0:48] = lt
    talogf[48:96, 48:96] = lt
    talogf[0:48, 96] = lt[:, END]
    talogf[48:96, 97] = lt[:, END]
    talogb = np.full((98, 96), NEG, f32)
    talogb[0:48, 0:48] = lt.T
    talogb[48:96, 48:96] = lt.T
    talogb[96, 0:48] = lt[:, END]
    talogb[97, 48:96] = lt[:, END]
    tendlog2 = np.full((2, 96), NEG, f32)
    tendlog2[0, 0:48] = lt[:, END]
    tendlog2[1, 48:96] = lt[:, END]
    rootlog2 = np.concatenate([lt[ROOT, :], lt[ROOT, :]]).astype(f32)[:, None]
    halfsel = np.zeros((96, 2), f32)
    halfsel[0:48, 0] = 1.0
    halfsel[48:96, 1] = 1.0

    # injection indicators: block i <-> L == 256 - i, i = 0..128
    Lh = L.reshape(2, H)                                    # [2, 32]
    ivals = 256 - np.arange(129)
    inj = (Lh[:, None, :] == ivals[None, :, None]).astype(f32)  # [2, 129, 32]
    inj = inj.reshape(2, 129 * H)

    # gold values (host-gathered, device-summed)
    bidx = np.arange(BL)
    tidx = np.arange(M)
    mask = tidx[None, :] < L[:, None]
    emitv = np.where(mask, feats_c[bidx[:, None], tidx[None, :], tg], 0.0)
    pmask = tidx[1:][None, :] < L[:, None]
    pairv = np.where(pmask, lt[tg[:, :-1], tg[:, 1:]], 0.0)
    goldv = np.zeros((BL, 516), f32)
    goldv[:, 0:256] = emitv
    goldv[:, 256:511] = pairv
    goldv[:, 511] = lt[ROOT, tg[:, 0]]
    goldv[:, 512] = lt[tg[bidx, L - 1], END]

    selw = np.zeros((BL, 128), f32)
    small = L <= 127
    selw[bidx[small], L[small] - 1] = 1.0
    selfb = np.stack([small, ~small], axis=1).astype(f32)
    lfl = L.astype(f32)[:, None]

    return {
        "efraw": ef,
        "ebraw": eb,
        "talogf": talogf,
        "talogb": talogb,
        "tendlog2": tendlog2,
        "rootlog2": rootlog2,
        "halfsel": halfsel,
        "injs": inj,
        "goldv": goldv,
        "selw": selw,
        "selfb": selfb,
        "lfl": lfl,
    }


def kernel(feats, tags, lengths, log_transitions):
    global _PROGRAM
    feats = np.asarray(feats, np.float32)
    tags = np.asarray(tags)
    lengths = np.asarray(lengths)
    lt = np.asarray(log_transitions, np.float32)

    in_maps = []
    for c in range(NC):
        sl = slice(c * BL, (c + 1) * BL)
        in_maps.append(_host_prep_core(feats[sl], tags[sl], lengths[sl], lt))

    if _PROGRAM is None:
        _PROGRAM = _build_program()

    res = run_bass_kernel_spmd(_PROGRAM, in_maps, core_ids=list(range(NC)))
    return np.concatenate([r["out"].reshape(BL) for r in res.results])


if __name__ == "__main__":
    rng = np.random.default_rng(0)
    feats = rng.standard_normal((B, M, T)).astype(np.float32)
    tags = rng.integers(0, ROOT, (B, M)).astype(np.int32)
    lengths = rng.integers(1, M + 1, (B,)).astype(np.int32)
    std = (2.0 / (T + T)) ** 0.5
    lt = (rng.standard_normal((T, T)) * std).astype(np.float32)
    lt[:, ROOT] = NEG
    lt[END, :] = NEG
    out = kernel(feats, tags, lengths, lt)
    print(out[:8], out.shape, out.dtype)
